# revision 8
# baseline (speedup 1.0000x reference)
"""CrossAttention kernel for 8 Trainium2 NeuronCores.

Reference computation (fp32):
    q = x @ Wq; k = ctx @ Wk; v = ctx @ Wv          (16 heads, d=64)
    out = softmax(q k^T / 8) v  reassembled, @ Wo + bo

Sharding: 8 cores = 2 batches x 4 head-groups (4 heads / 256 inner dims each).
Each core computes a partial output y_c = O_g @ Wo[group rows]; host sums the
4 partials per batch and adds bo.

Device-side layout trick: everything is computed transposed (S^T = K Q^T) so
the softmax reduction lands on the partition axis, where a ones-column
appended to V yields the denominators for free inside the P^T@V matmul.
x^T and ctx^T are prepared on host (layout prep only - all FLOPs on device).

Matmuls run as float32r (full PE rate at moving dim >= 256, ~fp32 accuracy).
"""

import numpy as np

import concourse.bass as bass
import concourse.mybir as mybir
import concourse.tile as tile
from concourse import bacc
from concourse.bass_utils import run_bass_kernel_spmd

HEADS = 16
DIM_HEAD = 64
B, N, M = 2, 2048, 1024
QDIM, CDIM = 1024, 768
INNER = HEADS * DIM_HEAD  # 1024
N_CORES = 8
GROUPS = 4                 # head-groups per batch
HLOC = HEADS // GROUPS     # 4 local heads per core
ILOC = HLOC * DIM_HEAD     # 256 local inner dims

f32 = mybir.dt.float32
f32r = mybir.dt.float32r
Exp = mybir.ActivationFunctionType.Exp

_CACHE = {}


def build_program():
    nc = bacc.Bacc("TRN2", target_bir_lowering=False, debug=False)

    xT = nc.dram_tensor("xT", [QDIM, N], f32r, kind="ExternalInput").ap()
    ctxT = nc.dram_tensor("ctxT", [CDIM, M], f32r, kind="ExternalInput").ap()
    wq = nc.dram_tensor("wq", [QDIM, ILOC], f32r, kind="ExternalInput").ap()
    wk = nc.dram_tensor("wk", [CDIM, ILOC], f32r, kind="ExternalInput").ap()
    wv = nc.dram_tensor("wv", [CDIM, ILOC], f32r, kind="ExternalInput").ap()
    wo = nc.dram_tensor("wo", [ILOC, QDIM], f32r, kind="ExternalInput").ap()
    y = nc.dram_tensor("y", [N, QDIM], f32, kind="ExternalOutput").ap()
    recip_dram = nc.dram_tensor("recip_dram", [8, 1024], f32).ap()

    QC = QDIM // 128   # 8 contraction chunks for Q proj
    CC = CDIM // 128   # 6 contraction chunks for K/V proj
    MT = M // 128      # 8 key tiles
    NT = N // 128      # 16 query tiles

    with tile.TileContext(nc) as tc:
        with (
            tc.tile_pool(name="consts", bufs=1) as consts,
            tc.tile_pool(name="xtp", bufs=2) as xtp,
            tc.tile_pool(name="ptp", bufs=3) as ptp,
            tc.tile_pool(name="smallp", bufs=2) as smallp,
            tc.tile_pool(name="ysbp", bufs=3) as ysbp,
        ):
            # ---- load weights + ctxT ----
            wq_sb = consts.tile([128, QC, ILOC], f32r)
            nc.sync.dma_start(wq_sb[:], wq.rearrange("(c p) d -> p c d", p=128))
            wk_sb = consts.tile([128, CC, ILOC], f32r)
            nc.sync.dma_start(wk_sb[:], wk.rearrange("(c p) d -> p c d", p=128))
            wv_sb = consts.tile([128, CC, ILOC], f32r)
            nc.sync.dma_start(wv_sb[:], wv.rearrange("(c p) d -> p c d", p=128))
            wo_sb = consts.tile([128, 2, QDIM], f32r)
            nc.sync.dma_start(wo_sb[:], wo.rearrange("(c p) d -> p c d", p=128))
            ctx_sb = consts.tile([128, CC, M], f32r)
            nc.sync.dma_start(ctx_sb[:], ctxT.rearrange("(c p) m -> p c m", p=128))

            kT_sb = [
                consts.tile([128, M], f32r, name=f"kT{s}") for s in range(2)
            ]
            qT_sb = [
                consts.tile([128, N], f32r, name=f"qT{s}") for s in range(2)
            ]
            vaug_sb = consts.tile([128, MT, HLOC, 65], f32r)
            ones_sb = consts.tile([128, MT * HLOC], f32)
            nc.vector.memset(ones_sb[:], 1.0)
            nc.vector.tensor_copy(vaug_sb[:, :, :, 64:65], ones_sb[:])
            oT_sb = [
                consts.tile([128, N], f32r, name=f"oT{p}") for p in range(2)
            ]

            # ---- phase A1/A2: K^T and V projections ----
            with tc.tile_pool(name="psA", space="PSUM", bufs=1) as psA:
                for s in range(2):
                    kps = psA.tile([128, M], f32, tag="k", bufs=2, name=f"kps{s}")
                    for c in range(CC):
                        for j in range(M // 512):
                            nc.tensor.matmul(
                                kps[:, j * 512:(j + 1) * 512],
                                wk_sb[:, c, s * 128:(s + 1) * 128],
                                ctx_sb[:, c, j * 512:(j + 1) * 512],
                                start=(c == 0),
                                stop=(c == CC - 1),
                            )
                    nc.vector.tensor_copy(kT_sb[s][:], kps[:])
                for mt in range(MT):
                    vps = psA.tile([128, ILOC], f32, tag="v", bufs=2, name=f"vps{mt}")
                    for c in range(CC):
                        nc.tensor.matmul(
                            vps[:],
                            ctx_sb[:, c, mt * 128:(mt + 1) * 128],
                            wv_sb[:, c, :],
                            start=(c == 0),
                            stop=(c == CC - 1),
                        )
                    for h in range(HLOC):
                        nc.vector.tensor_copy(
                            vaug_sb[:, mt, h, 0:64], vps[:, h * 64:(h + 1) * 64]
                        )

            # ---- phase A3: Q^T projection (x^T streamed once) ----
            with tc.tile_pool(name="psQ", space="PSUM", bufs=1) as psQ:
                qps = [
                    psQ.tile([128, N], f32, tag=f"q{s}", bufs=1, name=f"qps{s}")
                    for s in range(2)
                ]
                for c in range(QC):
                    xt = xtp.tile([128, N], f32r, tag="xt", name=f"xt{c}")
                    nc.sync.dma_start(xt[:], xT[c * 128:(c + 1) * 128, :])
                    for s in range(2):
                        for j in range(N // 512):
                            nc.tensor.matmul(
                                qps[s][:, j * 512:(j + 1) * 512],
                                wq_sb[:, c, s * 128:(s + 1) * 128],
                                xt[:, j * 512:(j + 1) * 512],
                                start=(c == 0),
                                stop=(c == QC - 1),
                            )
                for s in range(2):
                    nc.vector.tensor_copy(qT_sb[s][:], qps[s][:])

            # ---- phase B: attention, pairs x 1024-wide query strips ----
            with tc.tile_pool(name="psB", space="PSUM", bufs=1) as psB:
                for t in range(2):          # strip over queries
                    for p in range(2):      # head pair = qT/kT slice p
                        o_ps = [
                            psB.tile([128, 1024], f32, tag=f"o{h}", bufs=1,
                                     name=f"ops{t}{p}{h}")
                            for h in range(2)
                        ]
                        for mt in range(MT):
                            s_ps = psB.tile([128, 2048], f32, tag="s", bufs=1,
                                            name=f"sps{t}{p}{mt}")
                            for h in range(2):
                                lhsT = kT_sb[p][h * 64:(h + 1) * 64,
                                                mt * 128:(mt + 1) * 128]
                                for j in range(2):
                                    rhs = qT_sb[p][h * 64:(h + 1) * 64,
                                                   t * 1024 + j * 512:
                                                   t * 1024 + (j + 1) * 512]
                                    nc.tensor.matmul(
                                        s_ps[:, h * 1024 + j * 512:
                                             h * 1024 + (j + 1) * 512],
                                        lhsT, rhs,
                                        start=True, stop=True,
                                    )
                            pt = ptp.tile([128, 2048], f32r, tag="pt",
                                          name=f"pt{t}{p}{mt}")
                            nc.scalar.activation(
                                pt[:], s_ps[:], Exp, scale=DIM_HEAD ** -0.5
                            )
                            for h in range(2):
                                for j in range(2):
                                    nc.tensor.matmul(
                                        o_ps[h][0:65, j * 512:(j + 1) * 512],
                                        vaug_sb[:, mt, 2 * p + h, :],
                                        pt[:, h * 1024 + j * 512:
                                              h * 1024 + (j + 1) * 512],
                                        start=(mt == 0),
                                        stop=(mt == MT - 1),
                                    )
                        for h in range(2):
                            recip = smallp.tile([1, 1024], f32, tag="recip",
                                                name=f"rc{t}{p}{h}")
                            nc.vector.reciprocal(recip[:], o_ps[h][64:65, :])
                            rrep = smallp.tile([64, 1024], f32, tag="rrep",
                                               name=f"rr{t}{p}{h}")
                            idx = (t * 2 + p) * 2 + h
                            nc.sync.dma_start(
                                recip_dram[idx:idx + 1, :], recip[:]
                            )
                            bcast = bass.AP(
                                recip_dram.tensor, idx * 1024,
                                [[0, 64], [1, 1024]],
                            )
                            nc.sync.dma_start(rrep[:], bcast)
                            nc.vector.tensor_mul(
                                oT_sb[p][h * 64:(h + 1) * 64,
                                         t * 1024:(t + 1) * 1024],
                                o_ps[h][0:64, :],
                                rrep[:],
                            )

            # ---- phase C: output projection ----
            with tc.tile_pool(name="psC", space="PSUM", bufs=1) as psC:
                for it in range(NT):
                    yps = psC.tile([128, QDIM], f32, tag="y", bufs=2,
                                   name=f"yps{it}")
                    for p in range(2):
                        for j in range(QDIM // 512):
                            nc.tensor.matmul(
                                yps[:, j * 512:(j + 1) * 512],
                                oT_sb[p][:, it * 128:(it + 1) * 128],
                                wo_sb[:, p, j * 512:(j + 1) * 512],
                                start=(p == 0),
                                stop=(p == 1),
                            )
                    ysb = ysbp.tile([128, QDIM], f32, tag="ysb", name=f"ysb{it}")
                    nc.vector.tensor_copy(ysb[:], yps[:])
                    nc.sync.dma_start(y[it * 128:(it + 1) * 128, :], ysb[:])

    nc.compile()
    return nc


def make_in_maps(x, context, Wq, Wk, Wv, Wo):
    x = np.ascontiguousarray(np.asarray(x, dtype=np.float32))
    context = np.ascontiguousarray(np.asarray(context, dtype=np.float32))
    in_maps = []
    for c in range(N_CORES):
        b, g = divmod(c, GROUPS)
        sl = slice(g * ILOC, (g + 1) * ILOC)
        in_maps.append({
            "xT": np.ascontiguousarray(x[b].T),
            "ctxT": np.ascontiguousarray(context[b].T),
            "wq": np.ascontiguousarray(Wq[:, sl]),
            "wk": np.ascontiguousarray(Wk[:, sl]),
            "wv": np.ascontiguousarray(Wv[:, sl]),
            "wo": np.ascontiguousarray(Wo[sl, :]),
        })
    return in_maps


def kernel(x, context, Wq, Wk, Wv, Wo, bo, _results_out=None):
    if "nc" not in _CACHE:
        _CACHE["nc"] = build_program()
    nc = _CACHE["nc"]
    in_maps = make_in_maps(x, context, Wq, Wk, Wv, Wo)
    res = run_bass_kernel_spmd(nc, in_maps, list(range(N_CORES)))
    if _results_out is not None:
        _results_out.append(res)
    out = np.zeros((B, N, QDIM), dtype=np.float32)
    for c in range(N_CORES):
        b = c // GROUPS
        out[b] += res.results[c]["y"]
    out += np.asarray(bo, dtype=np.float32)[None, None, :]
    return out


# revision 10
# speedup vs baseline: 17062.3582x; 17062.3582x over previous
"""CrossAttention kernel for 8 Trainium2 NeuronCores.

Reference computation (fp32):
    q = x @ Wq; k = ctx @ Wk; v = ctx @ Wv          (16 heads, d=64)
    out = softmax(q k^T / 8) v  reassembled, @ Wo + bo

Sharding: 8 cores = 2 batches x 4 head-groups (4 heads / 256 inner dims each).
Each core computes a partial output y_c = O_g @ Wo[group rows]; host sums the
4 partials per batch and adds bo.

Device-side layout trick: everything is computed transposed (S^T = K Q^T) so
the softmax reduction lands on the partition axis, where a ones-column
appended to V yields the denominators for free inside the P^T@V matmul.
x^T and ctx^T are prepared on host (layout prep only - all FLOPs on device).

Matmuls run as float32r (full PE rate at moving dim >= 256, ~fp32 accuracy).
"""

import numpy as np

import concourse.bass as bass
import concourse.mybir as mybir
import concourse.tile as tile
from concourse import bacc
from concourse.bass_utils import run_bass_kernel_spmd

HEADS = 16
DIM_HEAD = 64
B, N, M = 2, 2048, 1024
QDIM, CDIM = 1024, 768
INNER = HEADS * DIM_HEAD  # 1024
N_CORES = 8
GROUPS = 4                 # head-groups per batch
HLOC = HEADS // GROUPS     # 4 local heads per core
ILOC = HLOC * DIM_HEAD     # 256 local inner dims

f32 = mybir.dt.float32
f32r = mybir.dt.float32r
Exp = mybir.ActivationFunctionType.Exp

_CACHE = {}


def build_program():
    nc = bacc.Bacc("TRN2", target_bir_lowering=False, debug=False)

    xT = nc.dram_tensor("xT", [QDIM, N], f32r, kind="ExternalInput").ap()
    ctxT = nc.dram_tensor("ctxT", [CDIM, M], f32r, kind="ExternalInput").ap()
    wq = nc.dram_tensor("wq", [QDIM, ILOC], f32r, kind="ExternalInput").ap()
    wk = nc.dram_tensor("wk", [CDIM, ILOC], f32r, kind="ExternalInput").ap()
    wv = nc.dram_tensor("wv", [CDIM, ILOC], f32r, kind="ExternalInput").ap()
    wo = nc.dram_tensor("wo", [ILOC, QDIM], f32r, kind="ExternalInput").ap()
    y = nc.dram_tensor("y", [N, QDIM], f32, kind="ExternalOutput").ap()
    recip_dram = nc.dram_tensor("recip_dram", [8, 1024], f32).ap()

    QC = QDIM // 128   # 8 contraction chunks for Q proj
    CC = CDIM // 128   # 6 contraction chunks for K/V proj
    MT = M // 128      # 8 key tiles
    NT = N // 128      # 16 query tiles

    with tile.TileContext(nc) as tc:
        with (
            tc.tile_pool(name="consts", bufs=1) as consts,
            tc.tile_pool(name="xtp", bufs=2) as xtp,
            tc.tile_pool(name="ptp", bufs=3) as ptp,
            tc.tile_pool(name="smallp", bufs=2) as smallp,
            tc.tile_pool(name="ysbp", bufs=4) as ysbp,
        ):
            # ---- load weights + ctxT (ctx + K/V weights first: K-proj
            # matmuls gate on them; per-chunk ctx DMAs let chunk-0 MMs
            # start before the whole 3MB lands) ----
            wk_sb = consts.tile([128, CC, ILOC], f32r)
            nc.sync.dma_start(wk_sb[:], wk.rearrange("(c p) d -> p c d", p=128))
            ctx_sb = consts.tile([128, CC, M], f32r)
            ctx_r = ctxT.rearrange("(c p) m -> p c m", p=128)
            for c in range(CC):
                nc.sync.dma_start(ctx_sb[:, c, :], ctx_r[:, c, :])
            wv_sb = consts.tile([128, CC, ILOC], f32r)
            nc.sync.dma_start(wv_sb[:], wv.rearrange("(c p) d -> p c d", p=128))
            wq_sb = consts.tile([128, QC, ILOC], f32r)
            nc.sync.dma_start(wq_sb[:], wq.rearrange("(c p) d -> p c d", p=128))
            wo_sb = consts.tile([128, 2, QDIM], f32r)
            nc.sync.dma_start(wo_sb[:], wo.rearrange("(c p) d -> p c d", p=128))

            kT_sb = [
                consts.tile([128, M], f32r, name=f"kT{s}") for s in range(2)
            ]
            qT_sb = [
                consts.tile([128, N], f32r, name=f"qT{s}") for s in range(2)
            ]
            vaug_sb = consts.tile([128, MT, HLOC, 65], f32r)
            ones_sb = consts.tile([128, MT * HLOC], f32)
            nc.vector.memset(ones_sb[:], 1.0)
            nc.vector.tensor_copy(vaug_sb[:, :, :, 64:65], ones_sb[:])
            oT_sb = [
                consts.tile([128, N], f32r, name=f"oT{p}") for p in range(2)
            ]

            # ---- phase A1/A2: K^T and V projections ----
            with tc.tile_pool(name="psA", space="PSUM", bufs=1) as psA:
                for s in range(2):
                    kps = psA.tile([128, M], f32, tag="k", bufs=2, name=f"kps{s}")
                    for c in range(CC):
                        for j in range(M // 512):
                            nc.tensor.matmul(
                                kps[:, j * 512:(j + 1) * 512],
                                wk_sb[:, c, s * 128:(s + 1) * 128],
                                ctx_sb[:, c, j * 512:(j + 1) * 512],
                                start=(c == 0),
                                stop=(c == CC - 1),
                            )
                    nc.vector.tensor_copy(kT_sb[s][:], kps[:])
                for mt in range(MT):
                    vps = psA.tile([128, ILOC], f32, tag="v", bufs=2, name=f"vps{mt}")
                    for c in range(CC):
                        nc.tensor.matmul(
                            vps[:],
                            ctx_sb[:, c, mt * 128:(mt + 1) * 128],
                            wv_sb[:, c, :],
                            start=(c == 0),
                            stop=(c == CC - 1),
                        )
                    for h in range(HLOC):
                        nc.vector.tensor_copy(
                            vaug_sb[:, mt, h, 0:64], vps[:, h * 64:(h + 1) * 64]
                        )

            # ---- phase A3: Q^T projection (x^T streamed once) ----
            with tc.tile_pool(name="psQ", space="PSUM", bufs=1) as psQ:
                qps = [
                    psQ.tile([128, N], f32, tag=f"q{s}", bufs=1, name=f"qps{s}")
                    for s in range(2)
                ]
                for c in range(QC):
                    xt = xtp.tile([128, N], f32r, tag="xt", name=f"xt{c}")
                    nc.sync.dma_start(xt[:], xT[c * 128:(c + 1) * 128, :])
                    for s in range(2):
                        for j in range(N // 512):
                            nc.tensor.matmul(
                                qps[s][:, j * 512:(j + 1) * 512],
                                wq_sb[:, c, s * 128:(s + 1) * 128],
                                xt[:, j * 512:(j + 1) * 512],
                                start=(c == 0),
                                stop=(c == QC - 1),
                            )
                for s in range(2):
                    nc.vector.tensor_copy(qT_sb[s][:], qps[s][:])

            # ---- phase B: attention, pairs x 1024-wide query strips ----
            with tc.tile_pool(name="psB", space="PSUM", bufs=1) as psB:
                for t in range(2):          # strip over queries
                    for p in range(2):      # head pair = qT/kT slice p
                        o_ps = [
                            psB.tile([128, 1024], f32, tag=f"o{h}", bufs=1,
                                     name=f"ops{t}{p}{h}")
                            for h in range(2)
                        ]
                        for mt in range(MT):
                            for h in range(2):
                                s_ps = psB.tile([128, 1024], f32, tag="s",
                                                bufs=2, name=f"sps{t}{p}{mt}{h}")
                                lhsT = kT_sb[p][h * 64:(h + 1) * 64,
                                                mt * 128:(mt + 1) * 128]
                                for j in range(2):
                                    rhs = qT_sb[p][h * 64:(h + 1) * 64,
                                                   t * 1024 + j * 512:
                                                   t * 1024 + (j + 1) * 512]
                                    nc.tensor.matmul(
                                        s_ps[:, j * 512:(j + 1) * 512],
                                        lhsT, rhs,
                                        start=True, stop=True,
                                    )
                                pt = ptp.tile([128, 1024], f32r, tag="pt",
                                              name=f"pt{t}{p}{mt}{h}")
                                nc.scalar.activation(
                                    pt[:], s_ps[:], Exp, scale=DIM_HEAD ** -0.5
                                )
                                for j in range(2):
                                    nc.tensor.matmul(
                                        o_ps[h][0:65, j * 512:(j + 1) * 512],
                                        vaug_sb[:, mt, 2 * p + h, :],
                                        pt[:, j * 512:(j + 1) * 512],
                                        start=(mt == 0),
                                        stop=(mt == MT - 1),
                                    )
                        for h in range(2):
                            recip = smallp.tile([1, 1024], f32, tag="recip",
                                                name=f"rc{t}{p}{h}")
                            nc.vector.reciprocal(recip[:], o_ps[h][64:65, :])
                            rrep = smallp.tile([64, 1024], f32, tag="rrep",
                                               name=f"rr{t}{p}{h}")
                            idx = (t * 2 + p) * 2 + h
                            nc.sync.dma_start(
                                recip_dram[idx:idx + 1, :], recip[:]
                            )
                            bcast = bass.AP(
                                recip_dram.tensor, idx * 1024,
                                [[0, 64], [1, 1024]],
                            )
                            nc.sync.dma_start(rrep[:], bcast)
                            nc.vector.tensor_mul(
                                oT_sb[p][h * 64:(h + 1) * 64,
                                         t * 1024:(t + 1) * 1024],
                                o_ps[h][0:64, :],
                                rrep[:],
                            )

            # ---- phase C: output projection ----
            with tc.tile_pool(name="psC", space="PSUM", bufs=1) as psC:
                for it in range(NT):
                    yps = psC.tile([128, QDIM], f32, tag="y", bufs=3,
                                   name=f"yps{it}")
                    for p in range(2):
                        for j in range(QDIM // 512):
                            nc.tensor.matmul(
                                yps[:, j * 512:(j + 1) * 512],
                                oT_sb[p][:, it * 128:(it + 1) * 128],
                                wo_sb[:, p, j * 512:(j + 1) * 512],
                                start=(p == 0),
                                stop=(p == 1),
                            )
                    ysb = ysbp.tile([128, QDIM], f32, tag="ysb", name=f"ysb{it}")
                    if it % 2 == 0:
                        nc.vector.tensor_copy(ysb[:], yps[:])
                    else:
                        nc.scalar.copy(ysb[:], yps[:])
                    nc.sync.dma_start(y[it * 128:(it + 1) * 128, :], ysb[:])

    nc.compile()
    return nc


def make_in_maps(x, context, Wq, Wk, Wv, Wo):
    x = np.ascontiguousarray(np.asarray(x, dtype=np.float32))
    context = np.ascontiguousarray(np.asarray(context, dtype=np.float32))
    in_maps = []
    for c in range(N_CORES):
        b, g = divmod(c, GROUPS)
        sl = slice(g * ILOC, (g + 1) * ILOC)
        in_maps.append({
            "xT": np.ascontiguousarray(x[b].T),
            "ctxT": np.ascontiguousarray(context[b].T),
            "wq": np.ascontiguousarray(Wq[:, sl]),
            "wk": np.ascontiguousarray(Wk[:, sl]),
            "wv": np.ascontiguousarray(Wv[:, sl]),
            "wo": np.ascontiguousarray(Wo[sl, :]),
        })
    return in_maps


def kernel(x, context, Wq, Wk, Wv, Wo, bo, _results_out=None):
    if "nc" not in _CACHE:
        _CACHE["nc"] = build_program()
    nc = _CACHE["nc"]
    in_maps = make_in_maps(x, context, Wq, Wk, Wv, Wo)
    res = run_bass_kernel_spmd(nc, in_maps, list(range(N_CORES)))
    if _results_out is not None:
        _results_out.append(res)
    out = np.zeros((B, N, QDIM), dtype=np.float32)
    for c in range(N_CORES):
        b = c // GROUPS
        out[b] += res.results[c]["y"]
    out += np.asarray(bo, dtype=np.float32)[None, None, :]
    return out


# revision 12
# speedup vs baseline: 17795.9810x; 1.0430x over previous
"""CrossAttention kernel for 8 Trainium2 NeuronCores.

Reference computation (fp32):
    q = x @ Wq; k = ctx @ Wk; v = ctx @ Wv          (16 heads, d=64)
    out = softmax(q k^T / 8) v  reassembled, @ Wo + bo

Sharding: 8 cores = 2 batches x 4 head-groups (4 heads / 256 inner dims each).
Each core computes a partial output y_c = O_g @ Wo[group rows]; host sums the
4 partials per batch and adds bo.

Device-side layout trick: everything is computed transposed (S^T = K Q^T) so
the softmax reduction lands on the partition axis, where a ones-column
appended to V yields the denominators for free inside the P^T@V matmul.
x^T and ctx^T are prepared on host (layout prep only - all FLOPs on device).

Matmuls run as float32r (full PE rate at moving dim >= 256, ~fp32 accuracy).
"""

import numpy as np

import concourse.bass as bass
import concourse.mybir as mybir
import concourse.tile as tile
from concourse import bacc
from concourse.bass_utils import run_bass_kernel_spmd

HEADS = 16
DIM_HEAD = 64
B, N, M = 2, 2048, 1024
QDIM, CDIM = 1024, 768
INNER = HEADS * DIM_HEAD  # 1024
N_CORES = 8
GROUPS = 4                 # head-groups per batch
HLOC = HEADS // GROUPS     # 4 local heads per core
ILOC = HLOC * DIM_HEAD     # 256 local inner dims

f32 = mybir.dt.float32
f32r = mybir.dt.float32r
Exp = mybir.ActivationFunctionType.Exp

_CACHE = {}


def build_program():
    nc = bacc.Bacc("TRN2", target_bir_lowering=False, debug=False)

    xT = nc.dram_tensor("xT", [QDIM, N], f32r, kind="ExternalInput").ap()
    ctxT = nc.dram_tensor("ctxT", [CDIM, M], f32r, kind="ExternalInput").ap()
    wq = nc.dram_tensor("wq", [QDIM, ILOC], f32r, kind="ExternalInput").ap()
    wk = nc.dram_tensor("wk", [CDIM, ILOC], f32r, kind="ExternalInput").ap()
    wv = nc.dram_tensor("wv", [CDIM, ILOC], f32r, kind="ExternalInput").ap()
    wo = nc.dram_tensor("wo", [ILOC, QDIM], f32r, kind="ExternalInput").ap()
    y = nc.dram_tensor("y", [N, QDIM], f32, kind="ExternalOutput").ap()

    QC = QDIM // 128   # 8 contraction chunks for Q proj
    CC = CDIM // 128   # 6 contraction chunks for K/V proj
    MT = M // 128      # 8 key tiles
    NT = N // 128      # 16 query tiles

    with tile.TileContext(nc) as tc:
        with (
            tc.tile_pool(name="consts", bufs=1) as consts,
            tc.tile_pool(name="xtp", bufs=2) as xtp,
            tc.tile_pool(name="ptp", bufs=3) as ptp,
            tc.tile_pool(name="smallp", bufs=2) as smallp,
            tc.tile_pool(name="ysbp", bufs=4) as ysbp,
        ):
            # ---- load weights + ctxT (ctx + K/V weights first: K-proj
            # matmuls gate on them; per-chunk ctx DMAs let chunk-0 MMs
            # start before the whole 3MB lands) ----
            wk_sb = consts.tile([128, CC, ILOC], f32r)
            nc.sync.dma_start(wk_sb[:], wk.rearrange("(c p) d -> p c d", p=128))
            ctx_sb = consts.tile([128, CC, M], f32r)
            ctx_r = ctxT.rearrange("(c p) m -> p c m", p=128)
            for c in range(CC):
                nc.sync.dma_start(ctx_sb[:, c, :], ctx_r[:, c, :])
            wv_sb = consts.tile([128, CC, ILOC], f32r)
            nc.sync.dma_start(wv_sb[:], wv.rearrange("(c p) d -> p c d", p=128))
            wq_sb = consts.tile([128, QC, ILOC], f32r)
            nc.sync.dma_start(wq_sb[:], wq.rearrange("(c p) d -> p c d", p=128))
            wo_sb = consts.tile([128, 2, QDIM], f32r)
            nc.sync.dma_start(wo_sb[:], wo.rearrange("(c p) d -> p c d", p=128))

            kT_sb = [
                consts.tile([128, M], f32r, name=f"kT{s}") for s in range(2)
            ]
            qT_sb = [
                consts.tile([128, N], f32r, name=f"qT{s}") for s in range(2)
            ]
            vaug_sb = consts.tile([128, MT, HLOC, 65], f32r)
            ones_sb = consts.tile([128, 64], f32)
            nc.vector.memset(ones_sb[:], 1.0)
            nc.vector.tensor_copy(vaug_sb[:, :, :, 64:65],
                                  ones_sb[:, 0:MT * HLOC])
            ones_r = consts.tile([1, 64], f32r)
            nc.vector.tensor_copy(ones_r[:], ones_sb[0:1, 0:64])
            oT_sb = [
                consts.tile([128, N], f32r, name=f"oT{p}") for p in range(2)
            ]

            # ---- phase A1/A2: K^T and V projections ----
            with tc.tile_pool(name="psA", space="PSUM", bufs=1) as psA:
                for s in range(2):
                    kps = psA.tile([128, M], f32, tag="k", bufs=2, name=f"kps{s}")
                    for c in range(CC):
                        for j in range(M // 512):
                            nc.tensor.matmul(
                                kps[:, j * 512:(j + 1) * 512],
                                wk_sb[:, c, s * 128:(s + 1) * 128],
                                ctx_sb[:, c, j * 512:(j + 1) * 512],
                                start=(c == 0),
                                stop=(c == CC - 1),
                            )
                    nc.vector.tensor_copy(kT_sb[s][:], kps[:])
                for mt in range(MT):
                    vps = psA.tile([128, ILOC], f32, tag="v", bufs=2, name=f"vps{mt}")
                    for c in range(CC):
                        nc.tensor.matmul(
                            vps[:],
                            ctx_sb[:, c, mt * 128:(mt + 1) * 128],
                            wv_sb[:, c, :],
                            start=(c == 0),
                            stop=(c == CC - 1),
                        )
                    for h in range(HLOC):
                        nc.vector.tensor_copy(
                            vaug_sb[:, mt, h, 0:64], vps[:, h * 64:(h + 1) * 64]
                        )

            # ---- phase A3: Q^T projection (x^T streamed once) ----
            with tc.tile_pool(name="psQ", space="PSUM", bufs=1) as psQ:
                qps = [
                    psQ.tile([128, N], f32, tag=f"q{s}", bufs=1, name=f"qps{s}")
                    for s in range(2)
                ]
                for c in range(QC):
                    xt = xtp.tile([128, N], f32r, tag="xt", name=f"xt{c}")
                    nc.sync.dma_start(xt[:], xT[c * 128:(c + 1) * 128, :])
                    for s in range(2):
                        for j in range(N // 512):
                            nc.tensor.matmul(
                                qps[s][:, j * 512:(j + 1) * 512],
                                wq_sb[:, c, s * 128:(s + 1) * 128],
                                xt[:, j * 512:(j + 1) * 512],
                                start=(c == 0),
                                stop=(c == QC - 1),
                            )
                for s in range(2):
                    nc.vector.tensor_copy(qT_sb[s][:], qps[s][:])

            # ---- phase B: attention, pairs x 1024-wide query strips ----
            with tc.tile_pool(name="psB", space="PSUM", bufs=1) as psB:
                for t in range(2):          # strip over queries
                    for p in range(2):      # head pair = qT/kT slice p
                        o_ps = [
                            psB.tile([128, 1024], f32, tag=f"o{h}", bufs=1,
                                     name=f"ops{t}{p}{h}")
                            for h in range(2)
                        ]
                        for mt in range(MT):
                            for h in range(2):
                                s_ps = psB.tile([128, 1024], f32, tag="s",
                                                bufs=2, name=f"sps{t}{p}{mt}{h}")
                                lhsT = kT_sb[p][h * 64:(h + 1) * 64,
                                                mt * 128:(mt + 1) * 128]
                                for j in range(2):
                                    rhs = qT_sb[p][h * 64:(h + 1) * 64,
                                                   t * 1024 + j * 512:
                                                   t * 1024 + (j + 1) * 512]
                                    nc.tensor.matmul(
                                        s_ps[:, j * 512:(j + 1) * 512],
                                        lhsT, rhs,
                                        start=True, stop=True,
                                    )
                                pt = ptp.tile([128, 1024], f32r, tag="pt",
                                              name=f"pt{t}{p}{mt}{h}")
                                nc.scalar.activation(
                                    pt[:], s_ps[:], Exp, scale=DIM_HEAD ** -0.5
                                )
                                for j in range(2):
                                    nc.tensor.matmul(
                                        o_ps[h][0:65, j * 512:(j + 1) * 512],
                                        vaug_sb[:, mt, 2 * p + h, :],
                                        pt[:, j * 512:(j + 1) * 512],
                                        start=(mt == 0),
                                        stop=(mt == MT - 1),
                                    )
                        for h in range(2):
                            recip = smallp.tile([1, 1024], f32, tag="recip",
                                                name=f"rc{t}{p}{h}")
                            nc.vector.reciprocal(recip[:], o_ps[h][64:65, :])
                            recip_r = smallp.tile([1, 1024], f32r, tag="recipr",
                                                  name=f"rcr{t}{p}{h}")
                            nc.vector.tensor_copy(recip_r[:], recip[:])
                            rrep_ps = psB.tile([64, 1024], f32, tag="s", bufs=2,
                                               name=f"rrp{t}{p}{h}")
                            for j in range(2):
                                nc.tensor.matmul(
                                    rrep_ps[:, j * 512:(j + 1) * 512],
                                    ones_r[:],
                                    recip_r[:, j * 512:(j + 1) * 512],
                                    start=True, stop=True,
                                )
                            rrep = smallp.tile([64, 1024], f32, tag="rrep",
                                               name=f"rr{t}{p}{h}")
                            nc.vector.tensor_copy(rrep[:], rrep_ps[:])
                            nc.vector.tensor_mul(
                                oT_sb[p][h * 64:(h + 1) * 64,
                                         t * 1024:(t + 1) * 1024],
                                o_ps[h][0:64, :],
                                rrep[:],
                            )

            # ---- phase C: output projection ----
            with tc.tile_pool(name="psC", space="PSUM", bufs=1) as psC:
                for it in range(NT):
                    yps = psC.tile([128, QDIM], f32, tag="y", bufs=3,
                                   name=f"yps{it}")
                    for p in range(2):
                        for j in range(QDIM // 512):
                            nc.tensor.matmul(
                                yps[:, j * 512:(j + 1) * 512],
                                oT_sb[p][:, it * 128:(it + 1) * 128],
                                wo_sb[:, p, j * 512:(j + 1) * 512],
                                start=(p == 0),
                                stop=(p == 1),
                            )
                    ysb = ysbp.tile([128, QDIM], f32, tag="ysb", name=f"ysb{it}")
                    if it % 2 == 0:
                        nc.vector.tensor_copy(ysb[:], yps[:])
                    else:
                        nc.scalar.copy(ysb[:], yps[:])
                    nc.sync.dma_start(y[it * 128:(it + 1) * 128, :], ysb[:])

    nc.compile()
    return nc


def make_in_maps(x, context, Wq, Wk, Wv, Wo):
    x = np.ascontiguousarray(np.asarray(x, dtype=np.float32))
    context = np.ascontiguousarray(np.asarray(context, dtype=np.float32))
    in_maps = []
    for c in range(N_CORES):
        b, g = divmod(c, GROUPS)
        sl = slice(g * ILOC, (g + 1) * ILOC)
        in_maps.append({
            "xT": np.ascontiguousarray(x[b].T),
            "ctxT": np.ascontiguousarray(context[b].T),
            "wq": np.ascontiguousarray(Wq[:, sl]),
            "wk": np.ascontiguousarray(Wk[:, sl]),
            "wv": np.ascontiguousarray(Wv[:, sl]),
            "wo": np.ascontiguousarray(Wo[sl, :]),
        })
    return in_maps


def kernel(x, context, Wq, Wk, Wv, Wo, bo, _results_out=None):
    if "nc" not in _CACHE:
        _CACHE["nc"] = build_program()
    nc = _CACHE["nc"]
    in_maps = make_in_maps(x, context, Wq, Wk, Wv, Wo)
    res = run_bass_kernel_spmd(nc, in_maps, list(range(N_CORES)))
    if _results_out is not None:
        _results_out.append(res)
    out = np.zeros((B, N, QDIM), dtype=np.float32)
    for c in range(N_CORES):
        b = c // GROUPS
        out[b] += res.results[c]["y"]
    out += np.asarray(bo, dtype=np.float32)[None, None, :]
    return out
